# revision 19
# baseline (speedup 1.0000x reference)
"""Multi-head attention (B=4, T=2048, D=1024, H=16) on 8 Trainium2 cores.

Sharding: batch (4-way) x head-half (2-way) -> 8 cores.
Core c handles batch b = c//2 and heads g*8..g*8+8 where g = c%2.

v2: single fully-interleaved phase. The attention inner loop is paced by
the ACT engine's exp (~2.0 us per (h,k) vs ~1.7 us of PE work), so all
QKV / out-proj GEMM groups are emitted as *fillers* between the
exp-dependent score/AV groups: the PE never stalls on the exp pipeline
and there is no serial GEMM prefix or out-proj tail.

  - No bias matmuls: q/k biases are folded into the psum->SBUF casts as
    per-partition tensor_scalar adds; the v bias is folded into the
    host-side epilogue (o+bv)@Wo^T = o@Wo^T + bv@Wo^T.
  - Filler schedule (deadline-driven): prefix = kT[j0] + q[j0] + v[t0..5];
    head 0: v[t6..15]; head 1: kT[j1]+q[j1]; heads 2-5: kT[j2],q[j2],
    kT[j3],q[j3]; heads 6-7: out-proj partials over jt0-2 (heads 0-5 are
    normalized by then). Tail = out-proj jt3 only + DVE add + DMA.
  - All GEMM/out-proj psums ride the scores psum tag's 2-slot rotation,
    so PSUM stays at 8 banks (scores 2x2 + AV 4).
  - Output DMA'd as bf16 (halved) and summed on host in f32.

Attention per head (unchanged from v1): scoresT[ki,qi] = kT2^T qpad
(K=128 with the other head's rows zeroed in qpad - keeps the PE activity
monitor at full clock), exp (scale=1/8) straight from psum in 1024-wide
ACT ops, AV: oT[j,qi] += [v|1|pad]^T wt with M=128 (row 64 = softmax
denom), AV(k-1) emitted after scores(k) (software pipeline). Normalize
via one [128,512] parallel reciprocal + DRAM-bounce partition-broadcast.
"""

import numpy as np
import ml_dtypes
from contextlib import ExitStack

import concourse.bass as bass
import concourse.tile as tile
from concourse import bacc, mybir
from concourse.bass_utils import run_bass_kernel_spmd

BF16_NP = ml_dtypes.bfloat16

B, T, D = 4, 2048, 1024
H, HD = 16, 64
P = 128
NC = 8
HPC = 8          # heads per core
JC = HPC * HD    # 512 head-dim columns per core
KT = D // P      # 8 contraction tiles for QKV
TT = T // P      # 16 t tiles
TCH = T // 512   # 4 t chunks of 512
F32 = mybir.dt.float32
BF16 = mybir.dt.bfloat16

_cached = {}


def build_program():
    nc = bacc.Bacc("TRN2", target_bir_lowering=False, debug=False,
                   enable_asserts=True, num_devices=NC)

    xt_d = nc.dram_tensor("xt", [TCH, P, KT, 512], BF16, kind="ExternalInput").ap()
    # j-piece major: piece jp in 0..7 (0-3 = q j-tiles, 4-7 = k j-tiles)
    wqk_d = nc.dram_tensor("wqk", [2 * JC // P, P, KT, P], BF16,
                           kind="ExternalInput").ap()
    wv_d = nc.dram_tensor("wv", [P, KT, JC], BF16, kind="ExternalInput").ap()
    bqk_d = nc.dram_tensor("bqk", [P, 2 * JC // P], F32,
                           kind="ExternalInput").ap()
    wo_d = nc.dram_tensor("wo", [P, JC // P, D], BF16, kind="ExternalInput").ap()
    out_d = nc.dram_tensor("out", [T, D], BF16, kind="ExternalOutput").ap()

    EXP = mybir.ActivationFunctionType.Exp
    VW = HPC * (HD + 1)
    NJP = 2 * JC // P    # 8 j pieces

    with tile.TileContext(nc) as tc:
        with ExitStack() as ctx:
            persist = ctx.enter_context(tc.tile_pool(name="persist", bufs=1))
            qk_sb = persist.tile([P, NJP, T], BF16, tag="qk")
            # [t, 8 heads x [v(64)|ones(1)]] + 64 pad cols so the AV
            # stationary operand can be sliced 128 wide
            vaug_f = persist.tile([P, TT, VW + HD], BF16, tag="vaug")
            ot_sb = persist.tile([P, JC // P, T], BF16, tag="ot")
            wo_sb = persist.tile([P, JC // P, D], BF16, tag="wo")
            bqk_sb = persist.tile([P, NJP], F32, tag="bqk")

            xt_sb = persist.tile([P, TCH, KT, 512], BF16, tag="xt")
            wqk_sb = persist.tile([P, NJP, KT, P], BF16, tag="wqk")
            wv_sb = persist.tile([P, KT, JC], BF16, tag="wv")
            opart = persist.tile([P, TT, D], BF16, tag="opart")
            # one static rb: consecutive heads use disjoint 64-row halves,
            # subtile deps let head h+2 overwrite after head h's multiply
            rb = persist.tile([P, T], F32, tag="rb")

            wtpool = ctx.enter_context(tc.tile_pool(name="wtpool", bufs=3))
            nrmpool = ctx.enter_context(tc.tile_pool(name="nrmpool", bufs=2))
            qpool = ctx.enter_context(tc.tile_pool(name="qpool", bufs=2))
            rdpool = ctx.enter_context(
                tc.tile_pool(name="rdpool", bufs=2, space="DRAM"))
            pss = ctx.enter_context(
                tc.tile_pool(name="pss", bufs=2, space="PSUM"))
            psav = ctx.enter_context(
                tc.tile_pool(name="psav", bufs=4, space="PSUM"))

            # input DMAs: x chunks on the sync queue, weights on gpsimd.
            # wqk piece order matches the filler schedule's deadlines.
            nc.gpsimd.dma_start(bqk_sb[:], bqk_d[:])
            for jp in (4, 0):
                nc.gpsimd.dma_start(wqk_sb[:, jp], wqk_d[jp])
            for tci in range(TCH):
                nc.sync.dma_start(xt_sb[:, tci], xt_d[tci])
            nc.gpsimd.dma_start(wv_sb[:], wv_d[:])
            for jp in (5, 1, 6, 2, 7, 3):
                nc.gpsimd.dma_start(wqk_sb[:, jp], wqk_d[jp])
            nc.gpsimd.dma_start(wo_sb[:], wo_d[:])

            vaug = vaug_f[:, :, 0:VW].rearrange(
                "p t (h e) -> p t h e", h=HPC)          # [128, 16, 8, 65]
            for tt in range(TT):
                nc.vector.memset(vaug[:, tt, :, HD:HD + 1], 1.0)
                nc.vector.memset(vaug_f[:, tt, VW:VW + HD], 1.0)

            # two rotating zero-padded qT buffers; heads alternate parity
            qpads = [qpool.tile([P, T], BF16, tag="qpad",
                                name=f"qpad_{i}") for i in range(2)]
            for i in range(2):
                nc.vector.memset(qpads[i][:], 0.0)

            # ---------------- filler GEMM groups ----------------
            def KG(j, tci):
                # kT j-tile j, chunk tci -> qk_sb[:, 4+j]
                tsl = slice(tci * 512, (tci + 1) * 512)
                ps = pss.tile([P, 512], F32, tag="ps_s", name=f"psk_{j}_{tci}")
                for k in range(KT):
                    nc.tensor.matmul(ps[:], wqk_sb[:, 4 + j, k, :],
                                     xt_sb[:, tci, k, :],
                                     start=(k == 0), stop=(k == KT - 1))
                nc.vector.tensor_scalar_add(
                    qk_sb[:, 4 + j, tsl], ps[:], bqk_sb[:, 4 + j:5 + j])

            def QG(j, tci):
                tsl = slice(tci * 512, (tci + 1) * 512)
                ps = pss.tile([P, 512], F32, tag="ps_s", name=f"psq_{j}_{tci}")
                for k in range(KT):
                    nc.tensor.matmul(ps[:], wqk_sb[:, j, k, :],
                                     xt_sb[:, tci, k, :],
                                     start=(k == 0), stop=(k == KT - 1))
                nc.vector.tensor_scalar_add(
                    qk_sb[:, j, tsl], ps[:], bqk_sb[:, j:j + 1])

            def VG(tt):
                tci, sub = tt // 4, tt % 4
                ps = pss.tile([P, 512], F32, tag="ps_s", name=f"psv_{tt}")
                for k in range(KT):
                    nc.tensor.matmul(
                        ps[:], xt_sb[:, tci, k, sub * P:(sub + 1) * P],
                        wv_sb[:, k, :],
                        start=(k == 0), stop=(k == KT - 1))
                nc.vector.tensor_copy(
                    vaug[:, tt, :, 0:HD],
                    ps[:].rearrange("p (h d) -> p h d", h=HPC))

            # out-proj partials over jt 0..2 (heads 0-5), emitted during
            # heads 6-7; the tail only contracts jt 3.
            def OP1(tt, cc):
                ps = pss.tile([P, 512], F32, tag="ps_s", name=f"pso1_{tt}_{cc}")
                for jt in range(3):
                    nc.tensor.matmul(
                        ps[:], ot_sb[:, jt, tt * P:(tt + 1) * P],
                        wo_sb[:, jt, cc * 512:(cc + 1) * 512],
                        start=(jt == 0), stop=(jt == 2))
                nc.vector.tensor_copy(
                    opart[:, tt, cc * 512:(cc + 1) * 512], ps[:])

            # ---------------- attention helpers ----------------
            def do_scores(h, k, qpad):
                jt = h // 2
                kT2 = qk_sb[:, jt + 4, :]
                wt = wtpool.tile([P, T], BF16, tag="wt", name=f"wt_{h}_{k}")
                for half in range(2):
                    ps = pss.tile([P, 2, 512], F32, tag="ps_s",
                                  name=f"ps_s_{h}_{k}_{half}")
                    for cc in range(2):
                        c4 = half * 2 + cc
                        nc.tensor.matmul(
                            ps[:, cc, :],
                            kT2[:, k * P:(k + 1) * P],
                            qpad[:, c4 * 512:(c4 + 1) * 512],
                            start=True, stop=True)
                    nc.scalar.activation(
                        wt[:, half * 1024:(half + 1) * 1024],
                        ps[:].rearrange("p a b -> p (a b)"),
                        EXP, bias=0.0, scale=0.125)
                return wt

            def do_av(h, k, wt, av_tiles):
                for c4 in range(4):
                    nc.tensor.matmul(
                        av_tiles[c4][:],
                        vaug_f[:, k, h * (HD + 1):h * (HD + 1) + P],
                        wt[:, c4 * 512:(c4 + 1) * 512],
                        start=(k == 0), stop=(k == TT - 1))

            def finish_head(h, av_tiles):
                # psum->SBUF copies + ~51-ULP reciprocal on the DVE; the
                # normalize multiply runs on the idle Pool engine so the
                # rb-DMA wait does not head-of-line-block the DVE queue
                pb = (h % 2) * 64
                jt = h // 2
                sums = nrmpool.tile([P, 512], F32, tag="sums",
                                    name=f"sums_{h}")
                for c4 in range(4):
                    csl = slice(c4 * 512, (c4 + 1) * 512)
                    nc.vector.tensor_copy(
                        ot_sb[pb:pb + 64, jt, csl],
                        av_tiles[c4][0:HD, :])
                    nc.vector.tensor_copy(
                        sums[32 * c4:32 * c4 + 1, :],
                        av_tiles[c4][HD:HD + 1, :])
                rcp = nrmpool.tile([P, 512], F32, tag="rcp", name=f"rcp_{h}")
                nc.vector.reciprocal(rcp[:], sums[:])
                rd = rdpool.tile([4, 512], F32, tag="rd", name=f"rd_{h}")
                nc.sync.dma_start(rd[:], rcp[0:128:32, :])
                rd_bcast = bass.AP(
                    tensor=rd.tensor, offset=rd.offset,
                    ap=[[0, 64], [512, 4], [1, 512]])
                nc.sync.dma_start(
                    rb[pb:pb + 64, :].rearrange("p (c r) -> p c r", c=4),
                    rd_bcast)
                nc.vector.tensor_mul(
                    ot_sb[pb:pb + 64, jt, :],
                    ot_sb[pb:pb + 64, jt, :],
                    rb[pb:pb + 64, :])

            def finish_head7_c4(av_tiles, c4):
                # head 7 finishes per 512-col chunk so the tail's first
                # out-proj groups can start ~5 us earlier
                csl = slice(c4 * 512, (c4 + 1) * 512)
                sums7, rcp7 = nrm7["sums"], nrm7["rcp"]
                nc.vector.tensor_copy(
                    ot_sb[64:128, 3, csl], av_tiles[c4][0:HD, :])
                nc.vector.tensor_copy(
                    sums7[32 * c4:32 * c4 + 1, :], av_tiles[c4][HD:HD + 1, :])
                nc.vector.reciprocal(
                    rcp7[32 * c4:32 * c4 + 1, :],
                    sums7[32 * c4:32 * c4 + 1, :])
                rd = rdpool.tile([1, 512], F32, tag="rd7", name=f"rd7_{c4}",
                                 bufs=4)
                nc.sync.dma_start(rd[:], rcp7[32 * c4:32 * c4 + 1, :])
                rd_bcast = bass.AP(
                    tensor=rd.tensor, offset=rd.offset,
                    ap=[[0, 64], [1, 512]])
                nc.sync.dma_start(rb[64:128, csl], rd_bcast)
                nc.vector.tensor_mul(
                    ot_sb[64:128, 3, csl], ot_sb[64:128, 3, csl],
                    rb[64:128, csl])

            # ---------------- filler schedule ----------------
            # prefix: everything head 0 needs + v for the first AV steps
            with nc.named_scope("prefix"):
                for tci in range(TCH):
                    KG(0, tci)
                    QG(0, tci)
                for tt in range(6):
                    VG(tt)

            # QG before KG within a head: the next head's qpad prefetch at
            # k=4 must find the q casts already emitted
            fillers = {h: [] for h in range(HPC)}
            fillers[0] = [(VG, (tt,)) for tt in range(6, TT)]
            fillers[1] = [(QG, (1, tci)) for tci in range(TCH)] + \
                         [(KG, (1, tci)) for tci in range(TCH)]
            fillers[2] = [(KG, (2, tci)) for tci in range(TCH)]
            fillers[3] = [(QG, (2, tci)) for tci in range(TCH)]
            fillers[4] = [(KG, (3, tci)) for tci in range(TCH)]
            fillers[5] = [(QG, (3, tci)) for tci in range(TCH)]
            fillers[6] = [(OP1, (tt, cc)) for tt in range(8)
                          for cc in range(2)]
            fillers[7] = [(OP1, (tt, cc)) for tt in range(8, TT)
                          for cc in range(2)]

            # head 0's qpad copy; later heads are prefetched mid-head on
            # the Pool engine so the PE never waits at a head boundary
            nc.vector.tensor_copy(qpads[0][0:HD, :], qk_sb[0:HD, 0, :])

            nrm7 = {"sums": nrmpool.tile([P, 512], F32, tag="sums7",
                                         name="sums_7", bufs=1),
                    "rcp": nrmpool.tile([P, 512], F32, tag="rcp7",
                                        name="rcp_7", bufs=1)}

            # ---------------- main interleaved loop ----------------
            prev = None
            for h in range(HPC):
                with nc.named_scope(f"head{h}"):
                    jt = h // 2
                    qpad = qpads[h % 2]
                    av_tiles = [psav.tile([P, 512], F32, tag="av",
                                          name=f"av_{h}_{i}")
                                for i in range(4)]
                    todo = list(fillers[h])
                    for k in range(TT):
                        wt = do_scores(h, k, qpad)
                        # at (6,0) head 5's finish_head is not yet emitted,
                        # and OP1 would read ot_sb[:, 2] before it is written
                        if todo and not (h == 6 and k == 0):
                            fn, args = todo.pop(0)
                            fn(*args)
                            # heads 6-7 carry 2 fillers per iteration
                            if h >= 6 and todo:
                                fn, args = todo.pop(0)
                                fn(*args)
                        if k == 4 and h + 1 < HPC:
                            nh = h + 1
                            npb = (nh % 2) * 64
                            nc.vector.tensor_copy(
                                qpads[nh % 2][npb:npb + HD, :],
                                qk_sb[npb:npb + HD, nh // 2, :])
                        if prev is not None:
                            ph, pk, pwt, pav = prev
                            do_av(ph, pk, pwt, pav)
                            if pk == TT - 1:
                                finish_head(ph, pav)
                        prev = (h, k, wt, av_tiles)
                    for fn, args in todo:
                        fn(*args)
            ph, pk, pwt, pav = prev
            do_av(ph, pk, pwt, pav)

            # ------- tail: head-7 finish + out-proj jt3, per c4 chunk -------
            with nc.named_scope("outtail"):
                opool = ctx.enter_context(tc.tile_pool(name="opool", bufs=2))
                for c4 in range(4):
                    finish_head7_c4(pav, c4)
                    for tt in range(c4 * 4, c4 * 4 + 4):
                        ost = opool.tile([P, D], BF16, tag="ost",
                                         name=f"ost_{tt}")
                        for cc in range(2):
                            ps = pss.tile([P, 512], F32, tag="ps_s",
                                          name=f"pso2_{tt}_{cc}")
                            nc.tensor.matmul(
                                ps[:], ot_sb[:, 3, tt * P:(tt + 1) * P],
                                wo_sb[:, 3, cc * 512:(cc + 1) * 512],
                                start=True, stop=True)
                            nc.vector.tensor_add(
                                ost[:, cc * 512:(cc + 1) * 512], ps[:],
                                opart[:, tt, cc * 512:(cc + 1) * 512])
                        eng = nc.sync if tt % 2 == 0 else nc.gpsimd
                        eng.dma_start(out_d[tt * P:(tt + 1) * P, :], ost[:])

    nc.compile()
    return nc


def _prep_core_inputs(x, qkv_w, qkv_b, out_w, core):
    b, g = core // 2, core % 2
    jsl = slice(g * JC, (g + 1) * JC)

    xT = np.ascontiguousarray(x[b].T)                       # [1024, 2048]
    xt = np.ascontiguousarray(
        xT.reshape(KT, P, TCH, 512).transpose(2, 1, 0, 3))  # [4, 128, 8, 512]

    Wq = qkv_w[0 * D:1 * D][jsl]                            # [512, 1024]
    Wk = qkv_w[1 * D:2 * D][jsl]
    Wv = qkv_w[2 * D:3 * D][jsl]
    WqkT = np.concatenate([Wq, Wk], axis=0).T               # [1024, 1024]
    # j-piece major: wqk[jp, p, k, m] = WqkT[k*128+p, jp*128+m]
    wqk = np.ascontiguousarray(
        WqkT.reshape(KT, P, 8, P).transpose(2, 1, 0, 3))    # [8, 128, 8, 128]
    WvT = Wv.T                                              # [1024, 512]
    wv = np.ascontiguousarray(
        WvT.reshape(KT, P, JC).transpose(1, 0, 2))          # [128, 8, 512]

    bqk_vec = np.concatenate(
        [qkv_b[0 * D:1 * D][jsl], qkv_b[1 * D:2 * D][jsl]])  # [1024]
    bqk = np.ascontiguousarray(bqk_vec.reshape(8, P).T)      # [128, 8]

    WoT = np.ascontiguousarray(out_w[:, jsl].T)             # [512, 1024]
    wo = np.ascontiguousarray(
        WoT.reshape(JC // P, P, D).transpose(1, 0, 2))      # [128, 4, 1024]

    return {
        "xt": xt.astype(BF16_NP),
        "wqk": wqk.astype(BF16_NP),
        "wv": wv.astype(BF16_NP),
        "bqk": bqk.astype(np.float32),
        "wo": wo.astype(BF16_NP),
    }


def run(x, qkv_w, qkv_b, out_w, out_b, trace=False, tmpdir=None):
    if "nc" not in _cached:
        _cached["nc"] = build_program()
    nc = _cached["nc"]
    in_maps = [_prep_core_inputs(x, qkv_w, qkv_b, out_w, c) for c in range(NC)]
    res = run_bass_kernel_spmd(nc, in_maps, core_ids=list(range(NC)),
                               trace=trace, tmpdir=tmpdir)
    parts = np.stack([np.asarray(res.results[c]["out"], dtype=np.float32)
                      for c in range(NC)])                  # [8, T, D]
    # v-bias epilogue: (o + bv) @ Wo^T = o @ Wo^T + bv @ Wo^T
    out_b_eff = qkv_b[2 * D:3 * D] @ out_w.T + out_b
    out = parts.reshape(B, 2, T, D).sum(axis=1) + out_b_eff[None, None, :]
    return out.astype(np.float32), res


def kernel(x, qkv_w, qkv_b, out_w, out_b):
    x = np.asarray(x, dtype=np.float32)
    qkv_w = np.asarray(qkv_w, dtype=np.float32)
    qkv_b = np.asarray(qkv_b, dtype=np.float32)
    out_w = np.asarray(out_w, dtype=np.float32)
    out_b = np.asarray(out_b, dtype=np.float32)
    out, _ = run(x, qkv_w, qkv_b, out_w, out_b, trace=False)
    return out


# revision 20
# speedup vs baseline: 1.0138x; 1.0138x over previous
"""Multi-head attention (B=4, T=2048, D=1024, H=16) on 8 Trainium2 cores.

Sharding: batch (4-way) x head-half (2-way) -> 8 cores.
Core c handles batch b = c//2 and heads g*8..g*8+8 where g = c%2.

v2: single fully-interleaved phase. The attention inner loop is paced by
the ACT engine's exp (~2.0 us per (h,k) vs ~1.7 us of PE work), so all
QKV / out-proj GEMM groups are emitted as *fillers* between the
exp-dependent score/AV groups: the PE never stalls on the exp pipeline
and there is no serial GEMM prefix or out-proj tail.

  - No bias matmuls: q/k biases are folded into the psum->SBUF casts as
    per-partition tensor_scalar adds; the v bias is folded into the
    host-side epilogue (o+bv)@Wo^T = o@Wo^T + bv@Wo^T.
  - Filler schedule (deadline-driven): prefix = kT[j0] + q[j0] + v[t0..5];
    head 0: v[t6..15]; head 1: kT[j1]+q[j1]; heads 2-5: kT[j2],q[j2],
    kT[j3],q[j3]; heads 6-7: out-proj partials over jt0-2 (heads 0-5 are
    normalized by then). Tail = out-proj jt3 only + DVE add + DMA.
  - All GEMM/out-proj psums ride the scores psum tag's 2-slot rotation,
    so PSUM stays at 8 banks (scores 2x2 + AV 4).
  - Output DMA'd as bf16 (halved) and summed on host in f32.

Attention per head (unchanged from v1): scoresT[ki,qi] = kT2^T qpad
(K=128 with the other head's rows zeroed in qpad - keeps the PE activity
monitor at full clock), exp (scale=1/8) straight from psum in 1024-wide
ACT ops, AV: oT[j,qi] += [v|1|pad]^T wt with M=128 (row 64 = softmax
denom), AV(k-1) emitted after scores(k) (software pipeline). Normalize
via one [128,512] parallel reciprocal + DRAM-bounce partition-broadcast.
"""

import numpy as np
import ml_dtypes
from contextlib import ExitStack

import concourse.bass as bass
import concourse.tile as tile
from concourse import bacc, mybir
from concourse.bass_utils import run_bass_kernel_spmd

BF16_NP = ml_dtypes.bfloat16

B, T, D = 4, 2048, 1024
H, HD = 16, 64
P = 128
NC = 8
HPC = 8          # heads per core
JC = HPC * HD    # 512 head-dim columns per core
KT = D // P      # 8 contraction tiles for QKV
TT = T // P      # 16 t tiles
TCH = T // 512   # 4 t chunks of 512
F32 = mybir.dt.float32
BF16 = mybir.dt.bfloat16

_cached = {}


def build_program():
    nc = bacc.Bacc("TRN2", target_bir_lowering=False, debug=False,
                   enable_asserts=True, num_devices=NC)

    xt_d = nc.dram_tensor("xt", [TCH, P, KT, 512], BF16, kind="ExternalInput").ap()
    # j-piece major: piece jp in 0..7 (0-3 = q j-tiles, 4-7 = k j-tiles)
    wqk_d = nc.dram_tensor("wqk", [2 * JC // P, P, KT, P], BF16,
                           kind="ExternalInput").ap()
    wv_d = nc.dram_tensor("wv", [P, KT, JC], BF16, kind="ExternalInput").ap()
    bqk_d = nc.dram_tensor("bqk", [P, 2 * JC // P], F32,
                           kind="ExternalInput").ap()
    wo_d = nc.dram_tensor("wo", [P, JC // P, D], BF16, kind="ExternalInput").ap()
    out_d = nc.dram_tensor("out", [T, D], BF16, kind="ExternalOutput").ap()

    EXP = mybir.ActivationFunctionType.Exp
    VW = HPC * (HD + 1)
    NJP = 2 * JC // P    # 8 j pieces

    with tile.TileContext(nc) as tc:
        with ExitStack() as ctx:
            persist = ctx.enter_context(tc.tile_pool(name="persist", bufs=1))
            qk_sb = persist.tile([P, NJP, T], BF16, tag="qk")
            # [t, 8 heads x [v(64)|ones(1)]] + 64 pad cols so the AV
            # stationary operand can be sliced 128 wide
            vaug_f = persist.tile([P, TT, VW + HD], BF16, tag="vaug")
            ot_sb = persist.tile([P, JC // P, T], BF16, tag="ot")
            wo_sb = persist.tile([P, JC // P, D], BF16, tag="wo")
            bqk_sb = persist.tile([P, NJP], F32, tag="bqk")

            xt_sb = persist.tile([P, TCH, KT, 512], BF16, tag="xt")
            wqk_sb = persist.tile([P, NJP, KT, P], BF16, tag="wqk")
            wv_sb = persist.tile([P, KT, JC], BF16, tag="wv")
            opart = persist.tile([P, TT, D], BF16, tag="opart")
            # one static rb: consecutive heads use disjoint 64-row halves,
            # subtile deps let head h+2 overwrite after head h's multiply
            rb = persist.tile([P, T], F32, tag="rb")

            wtpool = ctx.enter_context(tc.tile_pool(name="wtpool", bufs=3))
            nrmpool = ctx.enter_context(tc.tile_pool(name="nrmpool", bufs=2))
            qpool = ctx.enter_context(tc.tile_pool(name="qpool", bufs=2))
            rdpool = ctx.enter_context(
                tc.tile_pool(name="rdpool", bufs=2, space="DRAM"))
            pss = ctx.enter_context(
                tc.tile_pool(name="pss", bufs=2, space="PSUM"))
            psav = ctx.enter_context(
                tc.tile_pool(name="psav", bufs=4, space="PSUM"))

            # input DMAs: x chunks on the sync queue, weights on gpsimd.
            # wqk piece order matches the filler schedule's deadlines.
            nc.gpsimd.dma_start(bqk_sb[:], bqk_d[:])
            for jp in (4, 0):
                nc.gpsimd.dma_start(wqk_sb[:, jp], wqk_d[jp])
            for tci in range(TCH):
                nc.sync.dma_start(xt_sb[:, tci], xt_d[tci])
            nc.gpsimd.dma_start(wv_sb[:], wv_d[:])
            for jp in (5, 1, 6, 2, 7, 3):
                nc.gpsimd.dma_start(wqk_sb[:, jp], wqk_d[jp])
            nc.gpsimd.dma_start(wo_sb[:], wo_d[:])

            vaug = vaug_f[:, :, 0:VW].rearrange(
                "p t (h e) -> p t h e", h=HPC)          # [128, 16, 8, 65]
            for tt in range(TT):
                nc.vector.memset(vaug[:, tt, :, HD:HD + 1], 1.0)
                nc.vector.memset(vaug_f[:, tt, VW:VW + HD], 1.0)

            # two rotating zero-padded qT buffers; heads alternate parity
            qpads = [qpool.tile([P, T], BF16, tag="qpad",
                                name=f"qpad_{i}") for i in range(2)]
            for i in range(2):
                nc.vector.memset(qpads[i][:], 0.0)

            # ---------------- filler GEMM groups ----------------
            def KG(j, tci):
                # kT j-tile j, chunk tci -> qk_sb[:, 4+j]
                tsl = slice(tci * 512, (tci + 1) * 512)
                ps = pss.tile([P, 512], F32, tag="ps_s", name=f"psk_{j}_{tci}")
                for k in range(KT):
                    nc.tensor.matmul(ps[:], wqk_sb[:, 4 + j, k, :],
                                     xt_sb[:, tci, k, :],
                                     start=(k == 0), stop=(k == KT - 1))
                nc.vector.tensor_scalar_add(
                    qk_sb[:, 4 + j, tsl], ps[:], bqk_sb[:, 4 + j:5 + j])

            def QG(j, tci):
                tsl = slice(tci * 512, (tci + 1) * 512)
                ps = pss.tile([P, 512], F32, tag="ps_s", name=f"psq_{j}_{tci}")
                for k in range(KT):
                    nc.tensor.matmul(ps[:], wqk_sb[:, j, k, :],
                                     xt_sb[:, tci, k, :],
                                     start=(k == 0), stop=(k == KT - 1))
                nc.vector.tensor_scalar_add(
                    qk_sb[:, j, tsl], ps[:], bqk_sb[:, j:j + 1])

            def VG(tt):
                tci, sub = tt // 4, tt % 4
                ps = pss.tile([P, 512], F32, tag="ps_s", name=f"psv_{tt}")
                for k in range(KT):
                    nc.tensor.matmul(
                        ps[:], xt_sb[:, tci, k, sub * P:(sub + 1) * P],
                        wv_sb[:, k, :],
                        start=(k == 0), stop=(k == KT - 1))
                nc.vector.tensor_copy(
                    vaug[:, tt, :, 0:HD],
                    ps[:].rearrange("p (h d) -> p h d", h=HPC))

            # out-proj partials over jt 0..2 (heads 0-5), emitted during
            # heads 6-7; the tail only contracts jt 3.
            def OP1(tt, cc):
                ps = pss.tile([P, 512], F32, tag="ps_s", name=f"pso1_{tt}_{cc}")
                for jt in range(3):
                    nc.tensor.matmul(
                        ps[:], ot_sb[:, jt, tt * P:(tt + 1) * P],
                        wo_sb[:, jt, cc * 512:(cc + 1) * 512],
                        start=(jt == 0), stop=(jt == 2))
                nc.vector.tensor_copy(
                    opart[:, tt, cc * 512:(cc + 1) * 512], ps[:])

            # ---------------- attention helpers ----------------
            def do_scores(h, k, qpad):
                jt = h // 2
                kT2 = qk_sb[:, jt + 4, :]
                wt = wtpool.tile([P, T], BF16, tag="wt", name=f"wt_{h}_{k}")
                for half in range(2):
                    ps = pss.tile([P, 2, 512], F32, tag="ps_s",
                                  name=f"ps_s_{h}_{k}_{half}")
                    for cc in range(2):
                        c4 = half * 2 + cc
                        nc.tensor.matmul(
                            ps[:, cc, :],
                            kT2[:, k * P:(k + 1) * P],
                            qpad[:, c4 * 512:(c4 + 1) * 512],
                            start=True, stop=True)
                    nc.scalar.activation(
                        wt[:, half * 1024:(half + 1) * 1024],
                        ps[:].rearrange("p a b -> p (a b)"),
                        EXP, bias=0.0, scale=0.125)
                return wt

            def do_av(h, k, wt, av_tiles):
                for c4 in range(4):
                    nc.tensor.matmul(
                        av_tiles[c4][:],
                        vaug_f[:, k, h * (HD + 1):h * (HD + 1) + P],
                        wt[:, c4 * 512:(c4 + 1) * 512],
                        start=(k == 0), stop=(k == TT - 1))

            def finish_head(h, av_tiles):
                # psum->SBUF copies + ~51-ULP reciprocal on the DVE; the
                # normalize multiply runs on the idle Pool engine so the
                # rb-DMA wait does not head-of-line-block the DVE queue
                pb = (h % 2) * 64
                jt = h // 2
                sums = nrmpool.tile([P, 512], F32, tag="sums",
                                    name=f"sums_{h}")
                for c4 in range(4):
                    csl = slice(c4 * 512, (c4 + 1) * 512)
                    nc.vector.tensor_copy(
                        ot_sb[pb:pb + 64, jt, csl],
                        av_tiles[c4][0:HD, :])
                    nc.vector.tensor_copy(
                        sums[32 * c4:32 * c4 + 1, :],
                        av_tiles[c4][HD:HD + 1, :])
                rcp = nrmpool.tile([P, 512], F32, tag="rcp", name=f"rcp_{h}")
                nc.vector.reciprocal(rcp[:], sums[:])
                rd = rdpool.tile([4, 512], F32, tag="rd", name=f"rd_{h}")
                nc.sync.dma_start(rd[:], rcp[0:128:32, :])
                rd_bcast = bass.AP(
                    tensor=rd.tensor, offset=rd.offset,
                    ap=[[0, 64], [512, 4], [1, 512]])
                nc.sync.dma_start(
                    rb[pb:pb + 64, :].rearrange("p (c r) -> p c r", c=4),
                    rd_bcast)
                nc.gpsimd.tensor_mul(
                    ot_sb[pb:pb + 64, jt, :],
                    ot_sb[pb:pb + 64, jt, :],
                    rb[pb:pb + 64, :])

            # ---------------- filler schedule ----------------
            # prefix: everything head 0 needs + v for the first AV steps
            with nc.named_scope("prefix"):
                for tci in range(TCH):
                    KG(0, tci)
                    QG(0, tci)
                for tt in range(6):
                    VG(tt)

            # QG before KG within a head: the next head's qpad prefetch at
            # k=4 must find the q casts already emitted
            fillers = {h: [] for h in range(HPC)}
            fillers[0] = [(VG, (tt,)) for tt in range(6, TT)]
            fillers[1] = [(QG, (1, tci)) for tci in range(TCH)] + \
                         [(KG, (1, tci)) for tci in range(TCH)]
            fillers[2] = [(KG, (2, tci)) for tci in range(TCH)]
            fillers[3] = [(QG, (2, tci)) for tci in range(TCH)]
            fillers[4] = [(KG, (3, tci)) for tci in range(TCH)]
            fillers[5] = [(QG, (3, tci)) for tci in range(TCH)]
            fillers[6] = [(OP1, (tt, cc)) for tt in range(8)
                          for cc in range(2)]
            fillers[7] = [(OP1, (tt, cc)) for tt in range(8, TT)
                          for cc in range(2)]

            # head 0's qpad copy; later heads are prefetched mid-head on
            # the Pool engine so the PE never waits at a head boundary
            nc.gpsimd.tensor_copy(qpads[0][0:HD, :], qk_sb[0:HD, 0, :])

            # ---------------- main interleaved loop ----------------
            prev = None
            for h in range(HPC):
                with nc.named_scope(f"head{h}"):
                    jt = h // 2
                    qpad = qpads[h % 2]
                    av_tiles = [psav.tile([P, 512], F32, tag="av",
                                          name=f"av_{h}_{i}")
                                for i in range(4)]
                    todo = list(fillers[h])
                    for k in range(TT):
                        wt = do_scores(h, k, qpad)
                        # at (6,0) head 5's finish_head is not yet emitted,
                        # and OP1 would read ot_sb[:, 2] before it is written
                        if todo and not (h == 6 and k == 0):
                            fn, args = todo.pop(0)
                            fn(*args)
                            # heads 6-7 carry 2 fillers per iteration
                            if h >= 6 and todo:
                                fn, args = todo.pop(0)
                                fn(*args)
                        if k == 4 and h + 1 < HPC:
                            nh = h + 1
                            npb = (nh % 2) * 64
                            nc.gpsimd.tensor_copy(
                                qpads[nh % 2][npb:npb + HD, :],
                                qk_sb[npb:npb + HD, nh // 2, :])
                        if prev is not None:
                            ph, pk, pwt, pav = prev
                            do_av(ph, pk, pwt, pav)
                            if pk == TT - 1:
                                finish_head(ph, pav)
                        prev = (h, k, wt, av_tiles)
                    for fn, args in todo:
                        fn(*args)
            ph, pk, pwt, pav = prev
            do_av(ph, pk, pwt, pav)

            # ------- tail: head-7 finish + out-proj jt3 -------
            with nc.named_scope("outtail"):
                finish_head(ph, pav)
                opool = ctx.enter_context(tc.tile_pool(name="opool", bufs=2))
                for tt in range(TT):
                    ost = opool.tile([P, D], BF16, tag="ost",
                                     name=f"ost_{tt}")
                    for cc in range(2):
                        ps = pss.tile([P, 512], F32, tag="ps_s",
                                      name=f"pso2_{tt}_{cc}")
                        nc.tensor.matmul(
                            ps[:], ot_sb[:, 3, tt * P:(tt + 1) * P],
                            wo_sb[:, 3, cc * 512:(cc + 1) * 512],
                            start=True, stop=True)
                        nc.vector.tensor_add(
                            ost[:, cc * 512:(cc + 1) * 512], ps[:],
                            opart[:, tt, cc * 512:(cc + 1) * 512])
                    eng = nc.sync if tt % 2 == 0 else nc.gpsimd
                    eng.dma_start(out_d[tt * P:(tt + 1) * P, :], ost[:])

    nc.compile()
    return nc


def _prep_core_inputs(x, qkv_w, qkv_b, out_w, core):
    b, g = core // 2, core % 2
    jsl = slice(g * JC, (g + 1) * JC)

    xT = np.ascontiguousarray(x[b].T)                       # [1024, 2048]
    xt = np.ascontiguousarray(
        xT.reshape(KT, P, TCH, 512).transpose(2, 1, 0, 3))  # [4, 128, 8, 512]

    Wq = qkv_w[0 * D:1 * D][jsl]                            # [512, 1024]
    Wk = qkv_w[1 * D:2 * D][jsl]
    Wv = qkv_w[2 * D:3 * D][jsl]
    WqkT = np.concatenate([Wq, Wk], axis=0).T               # [1024, 1024]
    # j-piece major: wqk[jp, p, k, m] = WqkT[k*128+p, jp*128+m]
    wqk = np.ascontiguousarray(
        WqkT.reshape(KT, P, 8, P).transpose(2, 1, 0, 3))    # [8, 128, 8, 128]
    WvT = Wv.T                                              # [1024, 512]
    wv = np.ascontiguousarray(
        WvT.reshape(KT, P, JC).transpose(1, 0, 2))          # [128, 8, 512]

    bqk_vec = np.concatenate(
        [qkv_b[0 * D:1 * D][jsl], qkv_b[1 * D:2 * D][jsl]])  # [1024]
    bqk = np.ascontiguousarray(bqk_vec.reshape(8, P).T)      # [128, 8]

    WoT = np.ascontiguousarray(out_w[:, jsl].T)             # [512, 1024]
    wo = np.ascontiguousarray(
        WoT.reshape(JC // P, P, D).transpose(1, 0, 2))      # [128, 4, 1024]

    return {
        "xt": xt.astype(BF16_NP),
        "wqk": wqk.astype(BF16_NP),
        "wv": wv.astype(BF16_NP),
        "bqk": bqk.astype(np.float32),
        "wo": wo.astype(BF16_NP),
    }


def run(x, qkv_w, qkv_b, out_w, out_b, trace=False, tmpdir=None):
    if "nc" not in _cached:
        _cached["nc"] = build_program()
    nc = _cached["nc"]
    in_maps = [_prep_core_inputs(x, qkv_w, qkv_b, out_w, c) for c in range(NC)]
    res = run_bass_kernel_spmd(nc, in_maps, core_ids=list(range(NC)),
                               trace=trace, tmpdir=tmpdir)
    parts = np.stack([np.asarray(res.results[c]["out"], dtype=np.float32)
                      for c in range(NC)])                  # [8, T, D]
    # v-bias epilogue: (o + bv) @ Wo^T = o @ Wo^T + bv @ Wo^T
    out_b_eff = qkv_b[2 * D:3 * D] @ out_w.T + out_b
    out = parts.reshape(B, 2, T, D).sum(axis=1) + out_b_eff[None, None, :]
    return out.astype(np.float32), res


def kernel(x, qkv_w, qkv_b, out_w, out_b):
    x = np.asarray(x, dtype=np.float32)
    qkv_w = np.asarray(qkv_w, dtype=np.float32)
    qkv_b = np.asarray(qkv_b, dtype=np.float32)
    out_w = np.asarray(out_w, dtype=np.float32)
    out_b = np.asarray(out_b, dtype=np.float32)
    out, _ = run(x, qkv_w, qkv_b, out_w, out_b, trace=False)
    return out
